# revision 30
# baseline (speedup 1.0000x reference)
"""Single-head causal attention (B=8, T=2048, C=1024, H=64) on 8 TRN2 NeuronCores.

Strategy: pure data parallelism — batch element b runs on core b. Each core
computes, for its [T, C] slices q_b / k_b:

    Q = q_b @ Wq ; K = k_b @ Wk ; V = k_b @ Wv          (projections)
    S = Q @ K^T / sqrt(C), causal-masked ; P = exp(S)    (no max-subtract:
    out = (P @ V) / (P @ 1)                               S is well-scaled)

Device-side layout:
  * Host pre-transposes q/k to [C, T] and pre-blocks them [tb, p, c, t];
    contraction (C) lands on SBUF partitions, zero on-chip input transposes.
    k ships bf16 (feeds K and V);  q ships fp8-e4m3 and Wq ships fp8
    pre-scaled by 64 (1/64 folded into the exp scale), so the Q projection
    runs in DoubleRow mode (2 contraction chunks per matmul, 2x rate).
  * Weights+identity are DMA'd FIRST; ~3us of identity warm-up matmuls run
    while the first k block streams in, so the PE HAM clock-gate reaches
    8/8 before real work starts.  DMA order retires the k3-dependent tail
    last (q3 early), and attention for the last i-block is spread through
    the kernel into an SBUF accumulator so the endgame exp burst is gone.
  * Q projection uses a duplicated stationary [Wq | Wq], so Q^T comes out
    replicated on partition halves 0:64 / 64:128 in one pass.  K^T is
    likewise replicated to partitions 64:128 by a small SBUF->SBUF DMA.
  * Scores run as ROW-TILED PAIRS: chunk for key-tile j uses PE rows 0:63
    (operands on partitions 0:64), chunk j+1 rows 64:127 (operands on
    partitions 64:128); the two matmuls execute concurrently (contraction
    is only H=64), doubling score throughput.  One wide scalar-engine
    activation exps both PSUM banks of a pair.
  * Softmax denominators come free via a ones column appended to V (row 0
    of the PV accumulator is P @ 1).  Outputs leave PSUM by direct
    PSUM->DRAM DMA, unnormalized [l; out^T] fp32; the host divides.
"""

import numpy as np
import ml_dtypes

B, T, C, H = 8, 2048, 1024, 64
P = 128                  # SBUF partitions
CCH = C // P             # 8 contraction chunks
NJ = T // P              # 16 key tiles of 128
NB = T // 512            # 4 column blocks of 512
SCALE = float(C) ** -0.5
QS = 64.0                # fp8 Wq pre-scale (folded out via exp scale)
NWARM = 10               # HAM warm-up matmuls (512-col, scratch)

_cached = {}


def _enable_ldw_opt():
    """Let walrus hoist/dedupe LDWEIGHTS (off by default in this harness)."""
    import concourse.bass_utils as bu

    if getattr(bu, "_ldw_opt_patched", False):
        return
    orig = bu.run_command

    def run_command(cmd, *a, **kw):
        if isinstance(cmd, list):
            cmd = ["--enable-ldw-opt=true" if c == "--enable-ldw-opt=false" else c
                   for c in cmd]
        return orig(cmd, *a, **kw)

    bu.run_command = run_command
    bu._ldw_opt_patched = True


def _build():
    import concourse.bass as bass
    import concourse.mybir as mybir
    import concourse.tile as tile
    from concourse import bacc



    dt = mybir.dt
    nc = bacc.Bacc("TRN2", target_bir_lowering=False, debug=False, num_devices=B)

    qT = nc.dram_tensor("qT", [NB, P, CCH, 512], dt.float8e4, kind="ExternalInput").ap()
    kT = nc.dram_tensor("kT", [NB, P, CCH, 512], dt.bfloat16, kind="ExternalInput").ap()
    wq = nc.dram_tensor("wq", [P, CCH, P], dt.float8e4, kind="ExternalInput").ap()
    # packed bf16 consts: [wkv (CCH chunks) | shift | causal mask] — one DMA
    cb = nc.dram_tensor("cb", [P, CCH + 2, P], dt.bfloat16, kind="ExternalInput").ap()
    idb = nc.dram_tensor("idb", [P, P], dt.bfloat16, kind="ExternalInput").ap()
    # unnormalized [l ; out^T] per column block; host divides rows 1:65 by row 0
    out_t = nc.dram_tensor("out_t", [H + 1, T], dt.float32, kind="ExternalOutput").ap()
    # i-block 3 ships as 4 partial accumulations (host sums + normalizes)
    out3p = nc.dram_tensor("out3p", [4, H + 1, 512], dt.float32,
                           kind="ExternalOutput").ap()

    EXP = mybir.ActivationFunctionType.Exp
    DR = mybir.MatmulPerfMode.DoubleRow

    with tile.TileContext(nc) as tc:
        with (
            tc.tile_pool(name="consts", bufs=1) as consts,
            tc.tile_pool(name="inbuf", bufs=1) as inbuf,
            tc.tile_pool(name="proj", bufs=1) as proj,
            tc.tile_pool(name="projpsum", bufs=1, space="PSUM") as projpsum,
            tc.tile_pool(name="spsum", bufs=2, space="PSUM") as spsum,
            tc.tile_pool(name="opsum", bufs=1, space="PSUM") as opsum,
            tc.tile_pool(name="pbuf", bufs=3) as pbuf,
        ):
            # ---- constants: identity + weights FIRST on the sync ring -------
            idb_s = consts.tile([P, P], dt.bfloat16)
            cb_s = consts.tile([P, CCH + 2, P], dt.bfloat16)
            wq_s = consts.tile([P, CCH, P], dt.float8e4)
            wkv_s = cb_s[:, 0:CCH, :]
            nc.sync.dma_start(out=idb_s[:], in_=idb[:])

            scr_s = inbuf.tile([P, 512], dt.bfloat16)   # warm-up rhs
            nc.gpsimd.memset(scr_s[:], 0.0)
            kT_s = inbuf.tile([P, NB, CCH, 512], dt.bfloat16)
            qT_s = inbuf.tile([P, NB, CCH, 512], dt.float8e4)
            KVT_s = proj.tile([P, T], dt.bfloat16)   # rows 0:64 K^T, 64:128 V^T
            QT_s = proj.tile([P, T], dt.bfloat16)    # Q^T duplicated both halves
            KTD_s = proj.tile([P, T], dt.bfloat16)   # rows 64:128 = K^T dup
            V1_s = proj.tile([P, NJ, 66], dt.bfloat16)  # ones col + V natural
            nc.vector.memset(V1_s[:, :, 0:1], 1.0)

            # ---- input DMAs: consts ride the slow DMA ramp, then k0 halves --
            def dma_k(tb):
                nc.sync.dma_start(out=kT_s[:, tb, 0:4], in_=kT[tb, :, 0:4])
                nc.sync.dma_start(out=kT_s[:, tb, 4:8], in_=kT[tb, :, 4:8])

            def dma_q(tb):
                nc.sync.dma_start(out=qT_s[:, tb], in_=qT[tb, :])

            nc.sync.dma_start(out=kT_s[:, 0, 0:4], in_=kT[0, :, 0:4])
            nc.sync.dma_start(out=cb_s[:], in_=cb[:])
            nc.sync.dma_start(out=kT_s[:, 0, 4:8], in_=kT[0, :, 4:8])
            nc.sync.dma_start(out=wq_s[:], in_=wq[:])
            dma_q(0)
            dma_k(1)
            dma_q(1)
            dma_q(3)
            dma_k(2)
            dma_q(2)
            dma_k(3)

            # ---- HAM warm-up: identity matmuls while k0 streams -------------
            # (alternating PSUM banks so they issue back-to-back; the goal is
            # sustained PE activity so the clock gate reaches 8/8 before the
            # first real projection)
            for w in range(NWARM):
                wp = projpsum.tile([P, 512], dt.float32, tag="kv" if w % 2 else "q")
                nc.tensor.matmul(wp[:], lhsT=idb_s[:], rhs=scr_s[:],
                                 start=True, stop=True)

            # ---- pipeline stages --------------------------------------------
            def proj_kv_mms(tb):
                """KV projection matmuls as 4 chunks of 2 (interleavable)."""
                KVp = projpsum.tile([P, 512], dt.float32, tag="kv")

                def chunk(c2):
                    def emit():
                        for c in (2 * c2, 2 * c2 + 1):
                            nc.tensor.matmul(KVp[:], lhsT=wkv_s[:, c, :],
                                             rhs=kT_s[:, tb, c, :],
                                             start=(c == 0), stop=(c == CCH - 1))
                    return emit
                return KVp, [chunk(c2) for c2 in range(CCH // 2)]

            def proj_kv(tb, KVp=None, with_q=True):
                """Finish one k-block projection: copy K^T/V^T, optional Q,
                K^T dup shift, V transposes.  KVp=None emits the KV matmuls
                here; otherwise they were interleaved earlier."""
                sl = slice(512 * tb, 512 * (tb + 1))
                if KVp is None:
                    KVp, chunks = proj_kv_mms(tb)
                    for ch in chunks:
                        ch()
                nc.vector.tensor_copy(out=KVT_s[:, sl], in_=KVp[:])
                # Q projection here: its matmuls are independent of the KVT
                # copy, so they fill the PE while the DVE copy drains
                if with_q:
                    proj_q(tb)
                # replicate K^T onto partitions 64:128 for row-tiled scores:
                # PE shift-matmul (out[64+i,:] = K^T[i,:]) + small DVE copy —
                # a DMA here would crawl behind the streaming input transfers
                KDp = projpsum.tile([P, 512], dt.float32, tag="kv")
                nc.tensor.matmul(KDp[:], lhsT=cb_s[0:64, CCH, :],
                                 rhs=KVT_s[0:64, sl], start=True, stop=True)
                nc.vector.tensor_copy(out=KTD_s[64:128, sl], in_=KDp[64:128, :])
                for jj in range(4):
                    j = 4 * tb + jj
                    vtp = projpsum.tile([P, P], dt.bfloat16, tag="vt")
                    nc.tensor.transpose(
                        vtp[:], KVT_s[:, P * j:P * (j + 1)], idb_s[:])
                    nc.vector.tensor_copy(out=V1_s[:, j, 1:65], in_=vtp[:, 64:128])

            def proj_q(tb):
                """Project one 512-col block of q into Q^T (DoubleRow fp8)."""
                sl = slice(512 * tb, 512 * (tb + 1))
                Qp = projpsum.tile([P, 512], dt.float32, tag="q")
                for c2 in range(CCH // 2):
                    nc.tensor.matmul(Qp[:], lhsT=wq_s[:, 2 * c2:2 * c2 + 2, :],
                                     rhs=qT_s[:, tb, 2 * c2:2 * c2 + 2, :],
                                     perf_mode=DR,
                                     start=(c2 == 0), stop=(c2 == CCH // 2 - 1))
                nc.vector.tensor_copy(out=QT_s[:, sl], in_=Qp[:])

            def attn_pairs(ic, pairs, part_first, part_last, fillers=()):
                """Row-tiled score pairs + exp + PV accumulation for i-block ic.

                pairs: list of (jA, jB) or (jA, None).  part_first/part_last
                bound the PSUM accumulation group for this call."""
                ilo = 512 * ic
                ihi = 512 * (ic + 1)
                OUTp = opsum.tile([H + 1, 512], dt.float32, tag="out")

                def emit_pv(pv, is_first, is_last):
                    jA, jB, wA, wB, loA, loB, ob_, Pt = pv
                    nc.tensor.matmul(OUTp[:, loA - ilo:512],
                                     lhsT=V1_s[:, jA, 0:65],
                                     rhs=Pt[:, 0:wA],
                                     start=is_first, stop=False)
                    nc.tensor.matmul(OUTp[:, loB - ilo:512],
                                     lhsT=V1_s[:, jB, 0:65],
                                     rhs=Pt[:, ob_:ob_ + wB],
                                     start=False, stop=is_last)

                # scores run one pair AHEAD of PV so the in-order PE never
                # sits at a PV waiting for exp when the next S could run
                pend = None
                for pi, (jA, jB) in enumerate(pairs):
                    loA = max(P * jA, ilo)
                    loB = max(P * jB, ilo)
                    wA = ihi - loA
                    wB = ihi - loB
                    ob_ = 512   # B chunk in its own PSUM bank (concurrent drain)
                    Sp = spsum.tile([P, 1024], dt.float32, tag="s")
                    nc.tensor.matmul(Sp[:, 0:wA],
                                     lhsT=KVT_s[0:H, P * jA:P * (jA + 1)],
                                     rhs=QT_s[0:H, loA:loA + wA],
                                     start=True, stop=True)
                    nc.tensor.matmul(Sp[:, ob_:ob_ + wB],
                                     lhsT=KTD_s[64:128, P * jB:P * (jB + 1)],
                                     rhs=QT_s[64:128, loB:loB + wB],
                                     start=True, stop=True)
                    Pt = pbuf.tile([P, 1024], dt.bfloat16, tag="p")
                    nc.scalar.activation(out=Pt[:, 0:ob_ + wB],
                                         in_=Sp[:, 0:ob_ + wB],
                                         func=EXP, scale=SCALE / QS)
                    if jA >= 4 * ic:
                        nc.vector.tensor_mul(Pt[:, 0:P], Pt[:, 0:P],
                                             cb_s[:, CCH + 1, :])
                    if jB >= 4 * ic:
                        nc.vector.tensor_mul(Pt[:, ob_:ob_ + P],
                                             Pt[:, ob_:ob_ + P],
                                             cb_s[:, CCH + 1, :])
                    if pi < len(fillers):
                        fillers[pi]()
                    if pend is not None:
                        emit_pv(pend, part_first and pi == 1, False)
                    pend = (jA, jB, wA, wB, loA, loB, ob_, Pt)
                emit_pv(pend, part_first and len(pairs) == 1, part_last)
                return OUTp

            def attn_block(ic, fillers=()):
                """Full attention for i-block ic; unnormalized store via SBUF."""
                nj = 4 * ic + 4
                pairs = [(2 * p_, 2 * p_ + 1) for p_ in range(nj // 2)]
                OUTp = attn_pairs(ic, pairs, True, True, fillers)
                ob = pbuf.tile([H + 1, 512], dt.float32, tag="ob")
                nc.vector.tensor_copy(out=ob[:], in_=OUTp[:])
                nc.sync.dma_start(out=out_t[:, 512 * ic:512 * (ic + 1)],
                                  in_=ob[:])

            def attn3_part(pairs, pi, fillers=()):
                """Spread part of i-block 3: ship its partial sums; host adds."""
                OUTp = attn_pairs(3, pairs, True, True, fillers)
                ob = pbuf.tile([H + 1, 512], dt.float32, tag="ob")
                nc.vector.tensor_copy(out=ob[:], in_=OUTp[:])
                nc.sync.dma_start(out=out3p[pi], in_=ob[:])

            proj_kv(0)
            attn_block(0)
            proj_kv(1)
            proj_q(3)
            attn3_part([(0, 1), (2, 3)], 0)
            kvp2, f2 = proj_kv_mms(2)
            attn_block(1, fillers=f2)
            attn3_part([(4, 5), (6, 7)], 1)
            proj_kv(2, KVp=kvp2)
            kvp3, f3 = proj_kv_mms(3)
            attn_block(2, fillers=f3)
            attn3_part([(8, 9), (10, 11)], 2)
            proj_kv(3, KVp=kvp3, with_q=False)
            attn3_part([(12, 13), (14, 15)], 3)

    nc.compile()
    return nc


def _get_nc():
    if "nc" not in _cached:
        _cached["nc"] = _build()
    return _cached["nc"]


def _block(xT, dtype):
    """[C, T] -> [NB, P, CCH, 512] so each 512-col block is contiguous."""
    return np.ascontiguousarray(
        xT.reshape(CCH, P, NB, 512).transpose(2, 1, 0, 3)).astype(dtype)


def _wblock(w, dtype):
    """[C, Hw] -> [P, CCH, Hw] contiguous (contraction chunks on partitions)."""
    return np.ascontiguousarray(
        w.reshape(CCH, P, w.shape[1]).transpose(1, 0, 2)).astype(dtype)


def _host_inputs(q, k, Wq, Wk, Wv):
    bf16 = ml_dtypes.bfloat16
    fp8 = ml_dtypes.float8_e4m3
    wq_h = _wblock(np.concatenate([Wq, Wq], axis=1) * QS, fp8)
    wkv_h = _wblock(np.concatenate([Wk, Wv], axis=1), bf16)
    dmask_h = np.triu(np.ones((P, P), dtype=np.float32)).astype(bf16)
    idb_h = np.eye(P, dtype=np.float32).astype(bf16)
    shf_h = np.zeros((P, P), dtype=np.float32)
    shf_h[np.arange(64), 64 + np.arange(64)] = 1.0   # out[64+i] = in[i]
    shf_h = shf_h.astype(bf16)
    cb_h = np.concatenate(
        [wkv_h, shf_h[:, None, :], dmask_h[:, None, :]], axis=1)
    in_maps = []
    for b in range(B):
        in_maps.append({
            "qT": _block(q[b].T, fp8),
            "kT": _block(k[b].T, bf16),
            "wq": wq_h,
            "cb": cb_h,
            "idb": idb_h,
        })
    return in_maps


def _unshard(res_b):
    o = res_b["out_t"].copy()               # [H+1, T] f32: row 0 = l
    o[:, 1536:2048] = res_b["out3p"].sum(axis=0)
    return (o[1:H + 1] / o[0:1]).T          # [T, H]


def kernel(q, k, Wq, Wk, Wv):
    from concourse.bass_utils import run_bass_kernel_spmd

    nc = _get_nc()
    in_maps = _host_inputs(q, k, Wq, Wk, Wv)
    res = run_bass_kernel_spmd(nc, in_maps, list(range(B)))
    return np.stack([_unshard(res.results[b]) for b in range(B)]).astype(np.float32)


if __name__ == "__main__":
    rng = np.random.default_rng(0)
    q = rng.standard_normal((B, T, C)).astype(np.float32)
    k = rng.standard_normal((B, T, C)).astype(np.float32)
    Wq = (rng.standard_normal((C, H)) * 0.02).astype(np.float32)
    Wk = (rng.standard_normal((C, H)) * 0.02).astype(np.float32)
    Wv = (rng.standard_normal((C, H)) * 0.02).astype(np.float32)
    o = kernel(q, k, Wq, Wk, Wv)
    print("out", o.shape, o.dtype, float(np.abs(o).max()))


# revision 31
# speedup vs baseline: 1.1438x; 1.1438x over previous
"""Single-head causal attention (B=8, T=2048, C=1024, H=64) on 8 TRN2 NeuronCores.

Strategy: pure data parallelism — batch element b runs on core b. Each core
computes, for its [T, C] slices q_b / k_b:

    Q = q_b @ Wq ; K = k_b @ Wk ; V = k_b @ Wv          (projections)
    S = Q @ K^T / sqrt(C), causal-masked ; P = exp(S)    (no max-subtract:
    out = (P @ V) / (P @ 1)                               S is well-scaled)

Device-side layout:
  * Host pre-transposes q/k to [C, T] and pre-blocks them [tb, p, c, t];
    contraction (C) lands on SBUF partitions, zero on-chip input transposes.
    k ships bf16 (feeds K and V);  q ships fp8-e4m3 and Wq ships fp8
    pre-scaled by 64 (1/64 folded into the exp scale), so the Q projection
    runs in DoubleRow mode (2 contraction chunks per matmul, 2x rate).
  * Weights+identity are DMA'd FIRST; ~3us of identity warm-up matmuls run
    while the first k block streams in, so the PE HAM clock-gate reaches
    8/8 before real work starts.  DMA order retires the k3-dependent tail
    last (q3 early), and attention for the last i-block is spread through
    the kernel into an SBUF accumulator so the endgame exp burst is gone.
  * Q projection uses a duplicated stationary [Wq | Wq], so Q^T comes out
    replicated on partition halves 0:64 / 64:128 in one pass.  K^T is
    likewise replicated to partitions 64:128 by a small SBUF->SBUF DMA.
  * Scores run as ROW-TILED PAIRS: chunk for key-tile j uses PE rows 0:63
    (operands on partitions 0:64), chunk j+1 rows 64:127 (operands on
    partitions 64:128); the two matmuls execute concurrently (contraction
    is only H=64), doubling score throughput.  One wide scalar-engine
    activation exps both PSUM banks of a pair.
  * Softmax denominators come free via a ones column appended to V (row 0
    of the PV accumulator is P @ 1).  Outputs leave PSUM by direct
    PSUM->DRAM DMA, unnormalized [l; out^T] fp32; the host divides.
"""

import numpy as np
import ml_dtypes

B, T, C, H = 8, 2048, 1024, 64
P = 128                  # SBUF partitions
CCH = C // P             # 8 contraction chunks
NJ = T // P              # 16 key tiles of 128
NB = T // 512            # 4 column blocks of 512
SCALE = float(C) ** -0.5
QS = 64.0                # fp8 Wq pre-scale (folded out via exp scale)
NWARM = 18               # HAM warm-up matmuls (identity)

_cached = {}


def _enable_ldw_opt():
    """Let walrus hoist/dedupe LDWEIGHTS (off by default in this harness)."""
    import concourse.bass_utils as bu

    if getattr(bu, "_ldw_opt_patched", False):
        return
    orig = bu.run_command

    def run_command(cmd, *a, **kw):
        if isinstance(cmd, list):
            cmd = ["--enable-ldw-opt=true" if c == "--enable-ldw-opt=false" else c
                   for c in cmd]
        return orig(cmd, *a, **kw)

    bu.run_command = run_command
    bu._ldw_opt_patched = True


def _build():
    import concourse.bass as bass
    import concourse.mybir as mybir
    import concourse.tile as tile
    from concourse import bacc



    dt = mybir.dt
    nc = bacc.Bacc("TRN2", target_bir_lowering=False, debug=False, num_devices=B)

    qT = nc.dram_tensor("qT", [NB, P, CCH, 512], dt.float8e4, kind="ExternalInput").ap()
    kT = nc.dram_tensor("kT", [NB, P, CCH, 512], dt.bfloat16, kind="ExternalInput").ap()
    wq = nc.dram_tensor("wq", [P, CCH, P], dt.float8e4, kind="ExternalInput").ap()
    # packed bf16 consts: [wkv (CCH chunks) | shift | causal mask] — one DMA
    cb = nc.dram_tensor("cb", [P, CCH + 2, P], dt.bfloat16, kind="ExternalInput").ap()
    idb = nc.dram_tensor("idb", [P, P], dt.bfloat16, kind="ExternalInput").ap()
    # unnormalized [l ; out^T] per column block; host divides rows 1:65 by row 0
    out_t = nc.dram_tensor("out_t", [H + 1, T], dt.float32, kind="ExternalOutput").ap()
    # i-block 3 ships as 4 partial accumulations (host sums + normalizes)
    out3p = nc.dram_tensor("out3p", [4, H + 1, 512], dt.float32,
                           kind="ExternalOutput").ap()

    EXP = mybir.ActivationFunctionType.Exp
    DR = mybir.MatmulPerfMode.DoubleRow

    with tile.TileContext(nc) as tc:
        with (
            tc.tile_pool(name="consts", bufs=1) as consts,
            tc.tile_pool(name="inbuf", bufs=1) as inbuf,
            tc.tile_pool(name="proj", bufs=1) as proj,
            tc.tile_pool(name="projpsum", bufs=1, space="PSUM") as projpsum,
            tc.tile_pool(name="spsum", bufs=2, space="PSUM") as spsum,
            tc.tile_pool(name="opsum", bufs=1, space="PSUM") as opsum,
            tc.tile_pool(name="pbuf", bufs=3) as pbuf,
        ):
            # ---- constants: identity + weights FIRST on the sync ring -------
            idb_s = consts.tile([P, P], dt.bfloat16)
            cb_s = consts.tile([P, CCH + 2, P], dt.bfloat16)
            wq_s = consts.tile([P, CCH, P], dt.float8e4)
            wkv_s = cb_s[:, 0:CCH, :]
            nc.sync.dma_start(out=idb_s[:], in_=idb[:])

            kT_s = inbuf.tile([P, NB, CCH, 512], dt.bfloat16)
            qT_s = inbuf.tile([P, NB, CCH, 512], dt.float8e4)
            KVT_s = proj.tile([P, T], dt.bfloat16)   # rows 0:64 K^T, 64:128 V^T
            QT_s = proj.tile([P, T], dt.bfloat16)    # Q^T duplicated both halves
            KTD_s = proj.tile([P, T], dt.bfloat16)   # rows 64:128 = K^T dup
            V1_s = proj.tile([P, NJ, 66], dt.bfloat16)  # ones col + V natural
            nc.vector.memset(V1_s[:, :, 0:1], 1.0)

            # ---- input DMAs: consts ride the slow DMA ramp, then k0 halves --
            def dma_k(tb):
                nc.sync.dma_start(out=kT_s[:, tb, 0:4], in_=kT[tb, :, 0:4])
                nc.sync.dma_start(out=kT_s[:, tb, 4:8], in_=kT[tb, :, 4:8])

            def dma_q(tb):
                nc.sync.dma_start(out=qT_s[:, tb], in_=qT[tb, :])

            nc.sync.dma_start(out=kT_s[:, 0, 0:4], in_=kT[0, :, 0:4])
            nc.sync.dma_start(out=cb_s[:], in_=cb[:])
            nc.sync.dma_start(out=kT_s[:, 0, 4:8], in_=kT[0, :, 4:8])
            nc.sync.dma_start(out=wq_s[:], in_=wq[:])
            dma_q(0)
            dma_k(1)
            dma_q(1)
            dma_q(3)
            dma_k(2)
            dma_q(2)
            dma_k(3)

            # ---- HAM warm-up: identity matmuls while k0 streams -------------
            # (alternating PSUM banks so they issue back-to-back; the goal is
            # sustained PE activity so the clock gate reaches 8/8 before the
            # first real projection)
            for w in range(NWARM):
                wp = projpsum.tile([P, 512], dt.float32, tag="kv" if w % 2 else "q")
                nc.tensor.matmul(wp[:, 0:P], lhsT=idb_s[:], rhs=idb_s[:],
                                 start=True, stop=True)

            # ---- pipeline stages --------------------------------------------
            def proj_kv_mms(tb):
                """KV projection matmuls as 4 chunks of 2 (interleavable)."""
                KVp = projpsum.tile([P, 512], dt.float32, tag="kv")

                def chunk(c2):
                    def emit():
                        for c in (2 * c2, 2 * c2 + 1):
                            nc.tensor.matmul(KVp[:], lhsT=wkv_s[:, c, :],
                                             rhs=kT_s[:, tb, c, :],
                                             start=(c == 0), stop=(c == CCH - 1))
                    return emit
                return KVp, [chunk(c2) for c2 in range(CCH // 2)]

            def proj_kv(tb, KVp=None, with_q=True):
                """Finish one k-block projection: copy K^T/V^T, optional Q,
                K^T dup shift, V transposes.  KVp=None emits the KV matmuls
                here; otherwise they were interleaved earlier."""
                sl = slice(512 * tb, 512 * (tb + 1))
                if KVp is None:
                    KVp, chunks = proj_kv_mms(tb)
                    for ch in chunks:
                        ch()
                nc.vector.tensor_copy(out=KVT_s[:, sl], in_=KVp[:])
                # Q projection here: its matmuls are independent of the KVT
                # copy, so they fill the PE while the DVE copy drains
                if with_q:
                    proj_q(tb)
                # replicate K^T onto partitions 64:128 for row-tiled scores:
                # PE shift-matmul (out[64+i,:] = K^T[i,:]) + small DVE copy —
                # a DMA here would crawl behind the streaming input transfers
                KDp = projpsum.tile([P, 512], dt.float32, tag="kv")
                nc.tensor.matmul(KDp[:], lhsT=cb_s[0:64, CCH, :],
                                 rhs=KVT_s[0:64, sl], start=True, stop=True)
                nc.vector.tensor_copy(out=KTD_s[64:128, sl], in_=KDp[64:128, :])
                for jj in range(4):
                    j = 4 * tb + jj
                    vtp = projpsum.tile([P, P], dt.bfloat16, tag="vt")
                    nc.tensor.transpose(
                        vtp[:], KVT_s[:, P * j:P * (j + 1)], idb_s[:])
                    nc.vector.tensor_copy(out=V1_s[:, j, 1:65], in_=vtp[:, 64:128])

            def proj_q(tb):
                """Project one 512-col block of q into Q^T (DoubleRow fp8)."""
                sl = slice(512 * tb, 512 * (tb + 1))
                Qp = projpsum.tile([P, 512], dt.float32, tag="q")
                for c2 in range(CCH // 2):
                    nc.tensor.matmul(Qp[:], lhsT=wq_s[:, 2 * c2:2 * c2 + 2, :],
                                     rhs=qT_s[:, tb, 2 * c2:2 * c2 + 2, :],
                                     perf_mode=DR,
                                     start=(c2 == 0), stop=(c2 == CCH // 2 - 1))
                nc.vector.tensor_copy(out=QT_s[:, sl], in_=Qp[:])

            def attn_pairs(ic, pairs, part_first, part_last, fillers=()):
                """Row-tiled score pairs + exp + PV accumulation for i-block ic.

                pairs: list of (jA, jB) or (jA, None).  part_first/part_last
                bound the PSUM accumulation group for this call."""
                ilo = 512 * ic
                ihi = 512 * (ic + 1)
                OUTp = opsum.tile([H + 1, 512], dt.float32, tag="out")

                def emit_pv(pv, is_first, is_last):
                    jA, jB, wA, wB, loA, loB, ob_, Pt = pv
                    nc.tensor.matmul(OUTp[:, loA - ilo:512],
                                     lhsT=V1_s[:, jA, 0:65],
                                     rhs=Pt[:, 0:wA],
                                     start=is_first, stop=False)
                    nc.tensor.matmul(OUTp[:, loB - ilo:512],
                                     lhsT=V1_s[:, jB, 0:65],
                                     rhs=Pt[:, ob_:ob_ + wB],
                                     start=False, stop=is_last)

                # scores run one pair AHEAD of PV so the in-order PE never
                # sits at a PV waiting for exp when the next S could run
                pend = None
                for pi, (jA, jB) in enumerate(pairs):
                    loA = max(P * jA, ilo)
                    loB = max(P * jB, ilo)
                    wA = ihi - loA
                    wB = ihi - loB
                    ob_ = 512   # B chunk in its own PSUM bank (concurrent drain)
                    Sp = spsum.tile([P, 1024], dt.float32, tag="s")
                    nc.tensor.matmul(Sp[:, 0:wA],
                                     lhsT=KVT_s[0:H, P * jA:P * (jA + 1)],
                                     rhs=QT_s[0:H, loA:loA + wA],
                                     start=True, stop=True)
                    nc.tensor.matmul(Sp[:, ob_:ob_ + wB],
                                     lhsT=KTD_s[64:128, P * jB:P * (jB + 1)],
                                     rhs=QT_s[64:128, loB:loB + wB],
                                     start=True, stop=True)
                    Pt = pbuf.tile([P, 1024], dt.bfloat16, tag="p")
                    nc.scalar.activation(out=Pt[:, 0:ob_ + wB],
                                         in_=Sp[:, 0:ob_ + wB],
                                         func=EXP, scale=SCALE / QS)
                    if jA >= 4 * ic:
                        nc.vector.tensor_mul(Pt[:, 0:P], Pt[:, 0:P],
                                             cb_s[:, CCH + 1, :])
                    if jB >= 4 * ic:
                        nc.vector.tensor_mul(Pt[:, ob_:ob_ + P],
                                             Pt[:, ob_:ob_ + P],
                                             cb_s[:, CCH + 1, :])
                    if pi < len(fillers):
                        fillers[pi]()
                    if pend is not None:
                        emit_pv(pend, part_first and pi == 1, False)
                    pend = (jA, jB, wA, wB, loA, loB, ob_, Pt)
                emit_pv(pend, part_first and len(pairs) == 1, part_last)
                return OUTp

            def attn_block(ic, fillers=()):
                """Full attention for i-block ic; unnormalized store via SBUF."""
                nj = 4 * ic + 4
                pairs = [(2 * p_, 2 * p_ + 1) for p_ in range(nj // 2)]
                OUTp = attn_pairs(ic, pairs, True, True, fillers)
                ob = pbuf.tile([H + 1, 512], dt.float32, tag="ob")
                nc.vector.tensor_copy(out=ob[:], in_=OUTp[:])
                nc.sync.dma_start(out=out_t[:, 512 * ic:512 * (ic + 1)],
                                  in_=ob[:])

            def attn3_part(pairs, pi, fillers=()):
                """Spread part of i-block 3: ship its partial sums; host adds."""
                OUTp = attn_pairs(3, pairs, True, True, fillers)
                ob = pbuf.tile([H + 1, 512], dt.float32, tag="ob")
                nc.vector.tensor_copy(out=ob[:], in_=OUTp[:])
                nc.sync.dma_start(out=out3p[pi], in_=ob[:])

            proj_kv(0)
            attn_block(0)
            proj_kv(1)
            proj_q(3)
            attn3_part([(0, 1), (2, 3)], 0)
            attn_block(1)
            attn3_part([(4, 5), (6, 7)], 1)
            proj_kv(2)
            attn_block(2)
            attn3_part([(8, 9), (10, 11)], 2)
            proj_kv(3, with_q=False)
            attn3_part([(12, 13), (14, 15)], 3)

    nc.compile()
    return nc


def _get_nc():
    if "nc" not in _cached:
        _cached["nc"] = _build()
    return _cached["nc"]


def _block(xT, dtype):
    """[C, T] -> [NB, P, CCH, 512] so each 512-col block is contiguous."""
    return np.ascontiguousarray(
        xT.reshape(CCH, P, NB, 512).transpose(2, 1, 0, 3)).astype(dtype)


def _wblock(w, dtype):
    """[C, Hw] -> [P, CCH, Hw] contiguous (contraction chunks on partitions)."""
    return np.ascontiguousarray(
        w.reshape(CCH, P, w.shape[1]).transpose(1, 0, 2)).astype(dtype)


def _host_inputs(q, k, Wq, Wk, Wv):
    bf16 = ml_dtypes.bfloat16
    fp8 = ml_dtypes.float8_e4m3
    wq_h = _wblock(np.concatenate([Wq, Wq], axis=1) * QS, fp8)
    wkv_h = _wblock(np.concatenate([Wk, Wv], axis=1), bf16)
    dmask_h = np.triu(np.ones((P, P), dtype=np.float32)).astype(bf16)
    idb_h = np.eye(P, dtype=np.float32).astype(bf16)
    shf_h = np.zeros((P, P), dtype=np.float32)
    shf_h[np.arange(64), 64 + np.arange(64)] = 1.0   # out[64+i] = in[i]
    shf_h = shf_h.astype(bf16)
    cb_h = np.concatenate(
        [wkv_h, shf_h[:, None, :], dmask_h[:, None, :]], axis=1)
    in_maps = []
    for b in range(B):
        in_maps.append({
            "qT": _block(q[b].T, fp8),
            "kT": _block(k[b].T, bf16),
            "wq": wq_h,
            "cb": cb_h,
            "idb": idb_h,
        })
    return in_maps


def _unshard(res_b):
    o = res_b["out_t"].copy()               # [H+1, T] f32: row 0 = l
    o[:, 1536:2048] = res_b["out3p"].sum(axis=0)
    return (o[1:H + 1] / o[0:1]).T          # [T, H]


def kernel(q, k, Wq, Wk, Wv):
    from concourse.bass_utils import run_bass_kernel_spmd

    nc = _get_nc()
    in_maps = _host_inputs(q, k, Wq, Wk, Wv)
    res = run_bass_kernel_spmd(nc, in_maps, list(range(B)))
    return np.stack([_unshard(res.results[b]) for b in range(B)]).astype(np.float32)


if __name__ == "__main__":
    rng = np.random.default_rng(0)
    q = rng.standard_normal((B, T, C)).astype(np.float32)
    k = rng.standard_normal((B, T, C)).astype(np.float32)
    Wq = (rng.standard_normal((C, H)) * 0.02).astype(np.float32)
    Wk = (rng.standard_normal((C, H)) * 0.02).astype(np.float32)
    Wv = (rng.standard_normal((C, H)) * 0.02).astype(np.float32)
    o = kernel(q, k, Wq, Wk, Wv)
    print("out", o.shape, o.dtype, float(np.abs(o).max()))
